# revision 1
# baseline (speedup 1.0000x reference)
"""Trainium2 Bass kernel for nn_MCAF (dense_transformer).

Strategy: pure data-parallel over 8 NeuronCores (batch 131072 -> 16384/core).
Heavy host-side weight folding (uniform-attention collapse, V*O fusion, LN
scale/bias folding), bf16 front-end with HW DMA-transpose loads, fp32
transformer, feature-major on-chip layout (features on partitions, batch on
the free dim). ACT table-set thrash avoided with a wave structure.
"""

import sys

sys.path.insert(0, "/opt/trn_rl_repo")

import numpy as np
import ml_dtypes

import concourse.bass as bass
import concourse.bacc as bacc
import concourse.tile as tile
from concourse import mybir
from concourse.bass_utils import run_bass_kernel_spmd

BF16 = ml_dtypes.bfloat16
F32 = np.float32

B_TOTAL = 131072
N_CORES = 8
B_CORE = B_TOTAL // N_CORES          # 16384
SUB = 512                            # batch columns per sub-tile
N_SUB = B_CORE // SUB                # 32
N_PAIR = N_SUB // 2                  # 16
XE_COLS = 384                        # 5 l-chunks*64 + eog 48 + pad 16
AF = mybir.ActivationFunctionType
ALU = mybir.AluOpType
dt = mybir.dt


# ---------------------------------------------------------------- host folding
def _fold_weights(w):
    """Returns (wbf [128,NBF] bf16 blob, wf32 [128,NF32] f32 blob, col index map)."""
    eeg_ow = w["eeg_ow"].astype(np.float64)
    wv = w["eeg_inw"][124:186].astype(np.float64)
    bv = w["eeg_inb"][124:186].astype(np.float64)
    Me5 = (eeg_ow @ wv) / 5.0                        # [62,62]
    c_e = eeg_ow @ bv + w["eeg_ob"].astype(np.float64)

    # --- bf16 blob ---
    NBF = 3 * 128 + 3 * 32 + 2 * 32 + 128            # obar, h, og(raw,alpha), eluW
    wbf = np.zeros((128, NBF), np.float64)
    ob_off = 0
    h_off = 3 * 128
    ogr_off = h_off + 3 * 32
    ogo_off = ogr_off + 32
    elu_off = ogo_off + 32

    # obar blocks: x320 row g=64*l+c ; M=128 cols: [obar(62) 0 0 | obar(62) 0 0]
    for t in range(3):
        rows = 64 if t == 2 else 128
        blk = np.zeros((128, 128), np.float64)
        for r in range(rows):
            g = 128 * t + r
            l, c = g // 64, g % 64
            if c < 62:
                blk[r, 0:62] = Me5[:, c]
                blk[r, 64:126] = Me5[:, c]
            elif g == 62:                            # host plants 1.0 in x320[:,62]
                blk[r, 0:62] = c_e
                blk[r, 64:126] = c_e
        wbf[:, ob_off + 128 * t: ob_off + 128 * (t + 1)] = blk

    # h blocks: contraction over y (same row layout), M=32
    cw = w["eeg_cw"].astype(np.float64)              # [32,62,5]
    for t in range(3):
        rows = 64 if t == 2 else 128
        blk = np.zeros((128, 32), np.float64)
        for r in range(rows):
            g = 128 * t + r
            l, c = g // 64, g % 64
            if c < 62:
                blk[r, :] = cw[:, c, l]
        wbf[:, h_off + 32 * t: h_off + 32 * (t + 1)] = blk

    # eog blocks live at partitions 64..111 (xo data sits at rows 64:112 of t2)
    alpha33 = float(w["eog_inw"][2, 0]) * float(w["eog_ow"][0, 0]) / 33.0
    beta = float(w["eog_inb"][2]) * float(w["eog_ow"][0, 0]) + float(w["eog_ob"][0])
    cwo = w["eog_cw"].astype(np.float64)             # [32,1,33]
    raw_blk = np.zeros((128, 32), np.float64)
    alp_blk = np.zeros((128, 32), np.float64)
    for l in range(33):
        raw_blk[64 + l, :] = cwo[:, 0, l]
        alp_blk[64 + l, :] = alpha33
    alp_blk[64 + 33, :] = beta                       # host plants 1.0 in xo[:,33]
    wbf[:, ogr_off: ogr_off + 32] = raw_blk
    wbf[:, ogo_off: ogo_off + 32] = alp_blk

    # fused (ef,of)->feat weights, elu stack rows: [eeg32, og32] per half
    fw = w["fus_w"].astype(np.float64)               # [64,128]
    W_e = fw[:, :64] @ w["eeg_fw"].astype(np.float64)    # [64,32]
    W_o = fw[:, 64:] @ w["eog_fw"].astype(np.float64)    # [64,32]
    elu_blk = np.zeros((128, 128), np.float64)
    elu_blk[0:32, 0:64] = W_e.T
    elu_blk[32:64, 0:64] = W_o.T
    elu_blk[64:96, 64:128] = W_e.T
    elu_blk[96:128, 64:128] = W_o.T
    wbf[:, elu_off: elu_off + 128] = elu_blk

    # --- f32 blob ---
    cols = {}
    blocks = []

    def add(name, arr):                              # arr [128, n]
        cols[name] = sum(b.shape[1] for b in blocks)
        blocks.append(arr)

    I128 = np.eye(128)
    add("I128", I128)

    def bdiag(blk):                                  # block-diag [128,128]
        out = np.zeros((128, 128))
        out[0:64, 0:64] = blk
        out[64:128, 64:128] = blk
        return out

    C = np.eye(64) - 1.0 / 64.0
    add("center", bdiag(C))
    add("ones64", bdiag(np.ones((64, 64))))

    pe0 = (np.arange(64) % 2).astype(np.float64)
    b_feat = (fw[:, :64] @ w["eeg_fb"].astype(np.float64)
              + fw[:, 64:] @ w["eog_fb"].astype(np.float64)
              + w["fus_b"].astype(np.float64) + pe0
              - W_e.sum(axis=1) - W_o.sum(axis=1))   # fold elu's (e'-1)

    lay = []
    for i in range(2):
        s1 = w["tl_ln1_s"][i].astype(np.float64)
        b1v = w["tl_ln1_b"][i].astype(np.float64)
        Wvo = w["tl_ow"][i].astype(np.float64) @ w["tl_inw"][i, 128:192].astype(np.float64)
        bvo = (w["tl_ow"][i].astype(np.float64) @ w["tl_inb"][i, 128:192].astype(np.float64)
               + w["tl_ob"][i].astype(np.float64))
        Wvo_s = Wvo * s1[None, :]
        bvo_t = Wvo @ b1v + bvo
        s2 = w["tl_ln2_s"][i].astype(np.float64)
        b2v = w["tl_ln2_b"][i].astype(np.float64)
        W1 = w["tl_w1"][i].astype(np.float64)        # [256,64]
        W1_s = W1 * s2[None, :]
        b1g = W1 @ b2v + w["tl_b1"][i].astype(np.float64)   # [256]
        W2 = w["tl_w2"][i].astype(np.float64)        # [64,256]
        b2c = w["tl_b2"][i].astype(np.float64)
        lay.append((Wvo_s, bvo_t, W1_s, b1g, W2, b2c))
        add(f"attn{i}", bdiag(Wvo_s.T))
        m1 = W1_s.T                                  # [64,256]
        add(f"mlp1_{i}", np.concatenate([m1, m1], axis=0))
        add(f"mlp2_{i}", np.concatenate([W2.T[0:128], W2.T[128:256]], axis=1))  # [128,128]

    fn_s = w["fn_s"].astype(np.float64)
    fn_b = w["fn_b"].astype(np.float64)
    cls_w = w["cls_w"].astype(np.float64)
    cls_s = cls_w * fn_s[None, :]                    # [3,64]
    b_cls = cls_w @ fn_b + w["cls_b"].astype(np.float64)
    csT = cls_s.T                                    # [64,3]
    clsblk = np.zeros((128, 67))
    clsblk[0:64, 0:3] = csT
    clsblk[64:128, 64:67] = csT
    add("cls", clsblk)

    # per-partition bias columns
    def col(vals128):
        return np.asarray(vals128, np.float64).reshape(128, 1)

    cb_e = w["eeg_cb"].astype(np.float64)
    cb_o = w["eog_cb"].astype(np.float64)
    add("cbcol", col(np.concatenate([cb_e, cb_o, cb_e, cb_o])))
    add("bfeat", col(np.concatenate([b_feat, b_feat])))
    for i in range(2):
        add(f"bvo{i}", col(np.concatenate([lay[i][1], lay[i][1]])))
        add(f"b1gA{i}", col(lay[i][3][0:128]))
        add(f"b1gB{i}", col(lay[i][3][128:256]))
        add(f"b2c{i}", col(np.concatenate([lay[i][5], lay[i][5]])))
    bc = np.zeros(128)
    bc[0:3] = b_cls
    bc[64:67] = b_cls
    add("bcls", col(bc))
    add("eps", col(np.full(128, 1e-5)))
    add("zero", col(np.zeros(128)))

    wf32 = np.concatenate(blocks, axis=1)
    off = {"ob": ob_off, "h": h_off, "ogr": ogr_off, "ogo": ogo_off, "elu": elu_off}
    return wbf.astype(BF16), wf32.astype(F32), cols, off


# ---------------------------------------------------------------- device build
_CACHE = {}


def _build(nbf, nf32, cols, off):
    nc = bacc.Bacc("TRN2", target_bir_lowering=False, debug=False)
    xe_d = nc.dram_tensor("xe", [B_CORE, XE_COLS], dt.bfloat16, kind="ExternalInput")
    wbf_d = nc.dram_tensor("wbf", [128, nbf], dt.bfloat16, kind="ExternalInput")
    wf_d = nc.dram_tensor("wf32", [128, nf32], dt.float32, kind="ExternalInput")
    y_d = nc.dram_tensor("y_fm", [6, N_PAIR * SUB], dt.float32, kind="ExternalOutput")

    # persistent sbuf arrays
    wbf_sb = nc.alloc_sbuf_tensor("wbf_sb", [128, nbf], dt.bfloat16).ap()
    wf_sb = nc.alloc_sbuf_tensor("wf_sb", [128, nf32], dt.float32).ap()
    featA = nc.alloc_sbuf_tensor("featA", [128, N_PAIR * SUB], dt.float32).ap()
    featB = nc.alloc_sbuf_tensor("featB", [128, N_PAIR * SUB], dt.float32).ap()
    xn_sb = nc.alloc_sbuf_tensor("xn_sb", [128, N_PAIR * SUB], dt.float32).ap()
    out_fm = nc.alloc_sbuf_tensor("out_fm", [128, N_PAIR * SUB], dt.float32).ap()

    def W(name, r0=0, r1=128, n=None):
        c0 = cols[name]
        if n is None:
            n = {"I128": 128, "center": 128, "ones64": 128}.get(name, 64)
        return wf_sb[r0:r1, c0:c0 + n]

    def Wc(name, r0=0, r1=128):                      # bias columns
        return wf_sb[r0:r1, cols[name]:cols[name] + 1]

    with tile.TileContext(nc) as tc:
        nc.sync.dma_start(wbf_sb, wbf_d.ap())
        nc.sync.dma_start(wf_sb, wf_d.ap())

        # =================== PASS 1: front end (ACT set: exp) ===================
        with tc.tile_pool(name="p1sb", bufs=3) as sb, \
             tc.tile_pool(name="p1psA", bufs=2, space="PSUM") as psA, \
             tc.tile_pool(name="p1ps", bufs=1, space="PSUM") as ps:
            for p in range(N_PAIR):
                psH = psA.tile([128, SUB], dt.float32, tag="H")
                psF = ps.tile([128, SUB], dt.float32, tag="F")
                for h in range(2):
                    s = 2 * p + h
                    t0 = sb.tile([128, SUB], dt.bfloat16, tag="t0")
                    t1 = sb.tile([128, SUB], dt.bfloat16, tag="t1")
                    t2 = sb.tile([128, SUB], dt.bfloat16, tag="t2")
                    for t, tt in enumerate((t0, t1, t2)):
                        nc.sync.dma_start_transpose(
                            tt, xe_d.ap()[s * SUB:(s + 1) * SUB, 128 * t:128 * (t + 1)])
                    # obar (incl c_e bias via data ones-row)
                    psOB = psA.tile([128, SUB], dt.float32, tag="ob")
                    for t, tt in enumerate((t0, t1, t2)):
                        k = 64 if t == 2 else 128
                        nc.tensor.matmul(
                            psOB, wbf_sb[0:k, off["ob"] + 128 * t: off["ob"] + 128 * t + 128],
                            tt[0:k], start=(t == 0), stop=(t == 2))
                    obar2 = sb.tile([128, SUB], dt.bfloat16, tag="obar")
                    nc.scalar.activation(obar2, psOB, AF.Identity, bias=Wc("zero"))
                    # y = x * obar  (bf16)
                    y0 = sb.tile([128, SUB], dt.bfloat16, tag="y0")
                    y1 = sb.tile([128, SUB], dt.bfloat16, tag="y1")
                    y2 = sb.tile([128, SUB], dt.bfloat16, tag="y2")
                    nc.vector.tensor_mul(y0, t0, obar2)
                    nc.vector.tensor_mul(y1, t1, obar2)
                    nc.vector.tensor_mul(y2[0:64], t2[0:64], obar2[0:64])
                    # h (eeg) -> psH rows [0:32] / [64:96]
                    o = 64 * h
                    for t, yy in enumerate((y0, y1, y2)):
                        k = 64 if t == 2 else 128
                        nc.tensor.matmul(
                            psH[o:o + 32], wbf_sb[0:k, off["h"] + 32 * t: off["h"] + 32 * t + 32],
                            yy[0:k], start=(t == 0), stop=(t == 2))
                    # eog: raw + alpha*mean (+beta) -> base 32/96
                    o2 = 32 + 64 * h
                    psR = ps.tile([128, SUB], dt.float32, tag="ogr")
                    psO = ps.tile([128, SUB], dt.float32, tag="ogo")
                    nc.tensor.matmul(psR[o2:o2 + 32], wbf_sb[64:112, off["ogr"]:off["ogr"] + 32],
                                     t2[64:112], tile_position=(64, o2))
                    nc.tensor.matmul(psO[o2:o2 + 32], wbf_sb[64:112, off["ogo"]:off["ogo"] + 32],
                                     t2[64:112], tile_position=(64, o2))
                    o2sb = sb.tile([128, SUB], dt.float32, tag="o2sb")
                    nc.scalar.activation(o2sb[o2:o2 + 32], psO[o2:o2 + 32],
                                         AF.Identity, bias=Wc("zero", o2, o2 + 32))
                    nc.vector.tensor_mul(psH[o2:o2 + 32], psR[o2:o2 + 32], o2sb[o2:o2 + 32])
                # elu on stacked [eeg_lo, og_lo, eeg_hi, og_hi]
                r1 = sb.tile([128, SUB], dt.float32, tag="r1")
                sm = sb.tile([128, SUB], dt.float32, tag="sm")
                e1 = sb.tile([128, SUB], dt.float32, tag="e1")
                eluT = sb.tile([128, SUB], dt.bfloat16, tag="elu")
                nc.vector.tensor_scalar(r1, psH, Wc("cbcol"), 0.0, ALU.add, ALU.max)
                nc.vector.tensor_scalar(sm, psH, Wc("cbcol"), 0.0, ALU.add, ALU.min)
                nc.scalar.activation(e1, sm, AF.Exp, bias=Wc("zero"))
                nc.vector.tensor_add(eluT, r1, e1)
                # feat = W_elu.T @ elu (+b_feat), block-diag over halves
                nc.tensor.matmul(psF, wbf_sb[:, off["elu"]:off["elu"] + 128], eluT)
                nc.scalar.activation(featA[:, p * SUB:(p + 1) * SUB], psF,
                                     AF.Identity, bias=Wc("bfeat"))

        # =================== PASS 2: transformer ===================
        def layer_norm(ps, sb, src_ap, xn_out_ap):
            """src [128,SUB] sbuf -> xn (normalized, LN scale folded downstream)."""
            psXC = ps.tile([128, SUB], dt.float32, tag="a")
            nc.tensor.matmul(psXC, W("center"), src_ap)
            sq = sb.tile([128, SUB], dt.float32, tag="sq")
            nc.scalar.activation(sq, psXC, AF.Square, bias=Wc("zero"))
            psV = ps.tile([128, SUB], dt.float32, tag="b")
            nc.tensor.matmul(psV, W("ones64"), sq)
            sdev = sb.tile([128, SUB], dt.float32, tag="sd")
            nc.scalar.activation(sdev, psV, AF.Sqrt, bias=Wc("eps"), scale=1.0 / 64.0)
            rstd = sb.tile([128, SUB], dt.float32, tag="rs")
            nc.vector.reciprocal_approx_fast(rstd, sdev)
            nc.vector.tensor_mul(xn_out_ap, psXC, rstd)

        fsrc, fdst = featA, featB
        for i in range(2):
            # ---- wave 1: LN1 + attn + LN2  (ACT set: sqrt) ----
            tc.no_sync_barrier()
            with tc.tile_pool(name=f"w1s{i}", bufs=3) as sb, \
                 tc.tile_pool(name=f"w1c{i}", bufs=3, space="PSUM") as psc, \
                 tc.tile_pool(name=f"w1p{i}", bufs=2, space="PSUM") as ps:
                for p in range(N_PAIR):
                    sl = slice(p * SUB, (p + 1) * SUB)
                    xn1 = sb.tile([128, SUB], dt.float32, tag="xn1")
                    layer_norm(ps, sb, fsrc[:, sl], xn1)
                    psF2 = psc.tile([128, SUB], dt.float32, tag="c")
                    nc.tensor.matmul(psF2, W("I128", n=128), fsrc[:, sl],
                                     start=True, stop=False)
                    nc.tensor.matmul(psF2, W(f"attn{i}", n=128), xn1,
                                     start=False, stop=True)
                    nc.scalar.activation(fdst[:, sl], psF2, AF.Identity, bias=Wc(f"bvo{i}"))
                    layer_norm(ps, sb, fdst[:, sl], xn_sb[:, sl])
            # ---- wave 2: MLP (ACT set: gelu) ----
            tc.no_sync_barrier()
            with tc.tile_pool(name=f"w2s{i}", bufs=3) as sb, \
                 tc.tile_pool(name=f"w2c{i}", bufs=3, space="PSUM") as psc, \
                 tc.tile_pool(name=f"w2p{i}", bufs=2, space="PSUM") as ps:
                for p in range(N_PAIR):
                    sl = slice(p * SUB, (p + 1) * SUB)
                    gps, gsb = [], []
                    for hh in range(2):              # half: lo/hi
                        for ch in range(2):          # hidden chunk A/B
                            gp = ps.tile([128, SUB], dt.float32, tag="ab"[ch])
                            nc.tensor.matmul(
                                gp, W(f"mlp1_{i}", 64 * hh, 64 * hh + 64, n=256)
                                    [:, 128 * ch:128 * ch + 128],
                                xn_sb[64 * hh:64 * hh + 64, sl])
                            g = sb.tile([128, SUB], dt.float32, tag=f"g{hh}{ch}")
                            nc.scalar.activation(g, gp, AF.Gelu,
                                                 bias=Wc(f"b1g{'AB'[ch]}{i}"))
                            gps.append(gp)
                            gsb.append(g)
                    psF3 = psc.tile([128, SUB], dt.float32, tag="c")
                    nc.tensor.matmul(psF3, W("I128", n=128), fdst[:, sl],
                                     start=True, stop=False)
                    for hh in range(2):
                        for ch in range(2):
                            nc.tensor.matmul(
                                psF3[64 * hh:64 * hh + 64],
                                W(f"mlp2_{i}", n=128)[:, 64 * ch:64 * ch + 64],
                                gsb[2 * hh + ch], start=False,
                                stop=(hh == 1 and ch == 1))
                    nc.scalar.activation(fsrc[:, sl], psF3, AF.Identity, bias=Wc(f"b2c{i}"))
            # after layer: result lives in fsrc again (A -> B -> A)

        # ---- wave 3: final LN + classifier (ACT set: sqrt) ----
        tc.no_sync_barrier()
        with tc.tile_pool(name="w3s", bufs=3) as sb, \
             tc.tile_pool(name="w3c", bufs=3, space="PSUM") as psc, \
             tc.tile_pool(name="w3p", bufs=2, space="PSUM") as ps:
            for p in range(N_PAIR):
                sl = slice(p * SUB, (p + 1) * SUB)
                xn3 = sb.tile([128, SUB], dt.float32, tag="xn3")
                layer_norm(ps, sb, fsrc[:, sl], xn3)
                psO = psc.tile([128, SUB], dt.float32, tag="c")
                nc.tensor.matmul(psO[0:67], W("cls", n=67), xn3)
                nc.vector.tensor_scalar_add(out_fm[0:3, sl], psO[0:3], Wc("bcls", 0, 3))
                nc.vector.tensor_scalar_add(out_fm[64:67, sl], psO[64:67], Wc("bcls", 64, 67))
        nc.sync.dma_start(y_d.ap()[0:3, :], out_fm[0:3, :])
        nc.sync.dma_start(y_d.ap()[3:6, :], out_fm[64:67, :])

    nc.compile()
    return nc


# ---------------------------------------------------------------- entry point
def kernel(**inputs):
    w = {k: np.asarray(v) for k, v in inputs.items()}
    wbf, wf32, cols, off = _fold_weights(w)

    # x320 l-major: xe[:, 64*l + c] = eeg[:, c, l]; col 62 = 1.0 (c_e bias row)
    eeg = w["eeg"].astype(F32)
    xe = np.zeros((B_TOTAL, XE_COLS), F32)
    xe[:, 0:320].reshape(B_TOTAL, 5, 64)[:, :, 0:62] = eeg.transpose(0, 2, 1)
    xe[:, 62] = 1.0
    xe[:, 320:353] = w["eog"].astype(F32)[:, 0, :]
    xe[:, 353] = 1.0                                 # beta bias row (xo row 33)
    xe = xe.astype(BF16)

    key = ("prog", wbf.shape[1], wf32.shape[1])
    if key not in _CACHE:
        _CACHE[key] = _build(wbf.shape[1], wf32.shape[1], cols, off)
    nc = _CACHE[key]

    in_maps = []
    for k in range(N_CORES):
        in_maps.append({
            "xe": np.ascontiguousarray(xe[k * B_CORE:(k + 1) * B_CORE]),
            "wbf": wbf, "wf32": wf32,
        })
    res = run_bass_kernel_spmd(nc, in_maps, core_ids=list(range(N_CORES)))

    out = np.empty((B_TOTAL, 3), F32)
    for k in range(N_CORES):
        y = res.results[k]["y_fm"].reshape(2, 3, N_PAIR, SUB)
        out[k * B_CORE:(k + 1) * B_CORE] = (
            y.transpose(2, 0, 3, 1).reshape(B_CORE, 3))
    return out


if __name__ == "__main__":
    import reference
    ins = {k: np.asarray(v) for k, v in reference.setup_inputs().items()}
    got = kernel(**ins)
    exp = np.asarray(reference.reference(**ins))
    err = np.abs(got - exp).max() / (np.abs(exp).max() + 1e-9)
    print("Relative error:", err)



# revision 26
# speedup vs baseline: 2.2025x; 2.2025x over previous
"""Trainium2 Bass kernel for nn_MCAF (dense_transformer).

Strategy: pure data-parallel over 8 NeuronCores (batch 131072 -> 16384/core).
Heavy host-side weight folding (uniform-attention collapse, V*O fusion, LN
scale/bias folding), bf16 front-end with HW DMA-transpose loads, fp32r
(1 cycle/row PE) transformer, feature-major on-chip layout. Residual adds
fused into DVE scalar_tensor_tensor ops (no identity matmuls); LN rstd via
single Abs_reciprocal_sqrt activation; elementwise work spread across
ACT/DVE/Pool engines.
"""

import sys

sys.path.insert(0, "/opt/trn_rl_repo")

import numpy as np
import ml_dtypes

import concourse.bass as bass
import concourse.bacc as bacc
import concourse.tile as tile
from concourse import mybir
from concourse.bass_utils import run_bass_kernel_spmd

BF16 = ml_dtypes.bfloat16
F32 = np.float32

B_TOTAL = 131072
N_CORES = 8
B_CORE = B_TOTAL // N_CORES          # 16384
SUB = 512                            # batch columns per sub-tile
N_SUB = B_CORE // SUB                # 32
N_PAIR = N_SUB // 2                  # 16
XE_COLS = 384                        # 5 l-chunks*64 + eog 48 + pad 16
AF = mybir.ActivationFunctionType
ALU = mybir.AluOpType
dt = mybir.dt


# ---------------------------------------------------------------- host folding
def _fold_weights(w):
    """Returns (wbf [128,NBF] bf16 blob, wf32 [128,NF32] f32 blob, col maps).

    wf32 layout: matmul blocks first (consumed as float32r), then bias
    columns (consumed as float32). `mm_cols` is the boundary.
    """
    eeg_ow = w["eeg_ow"].astype(np.float64)
    wv = w["eeg_inw"][124:186].astype(np.float64)
    bv = w["eeg_inb"][124:186].astype(np.float64)
    Me5 = (eeg_ow @ wv) / 5.0                        # [62,62]
    c_e = eeg_ow @ bv + w["eeg_ob"].astype(np.float64)

    # --- bf16 blob ---
    # obar, h, og(raw,alpha), eluW, mlp2 x2, cls
    NBF = 3 * 128 + 3 * 32 + 2 * 32 + 128 + 2 * 128 + 67
    wbf = np.zeros((128, NBF), np.float64)
    ob_off = 0
    h_off = 3 * 128
    ogr_off = h_off + 3 * 32
    ogo_off = ogr_off + 32
    elu_off = ogo_off + 32
    mlp2_off = elu_off + 128
    cls_off = mlp2_off + 2 * 128

    # obar blocks: x320 row g=64*l+c ; M=128 cols: [obar(62) 0 0 | obar(62) 0 0]
    for t in range(3):
        rows = 64 if t == 2 else 128
        blk = np.zeros((128, 128), np.float64)
        for r in range(rows):
            g = 128 * t + r
            l, c = g // 64, g % 64
            if c < 62:
                blk[r, 0:62] = Me5[:, c]
                blk[r, 64:126] = Me5[:, c]
            elif g == 62:                            # host plants 1.0 in x320[:,62]
                blk[r, 0:62] = c_e
                blk[r, 64:126] = c_e
        wbf[:, ob_off + 128 * t: ob_off + 128 * (t + 1)] = blk

    # h blocks: contraction over y (same row layout), M=32
    cw = w["eeg_cw"].astype(np.float64)              # [32,62,5]
    for t in range(3):
        rows = 64 if t == 2 else 128
        blk = np.zeros((128, 32), np.float64)
        for r in range(rows):
            g = 128 * t + r
            l, c = g // 64, g % 64
            if c < 62:
                blk[r, :] = cw[:, c, l]
        wbf[:, h_off + 32 * t: h_off + 32 * (t + 1)] = blk

    # eog blocks live at partitions 64..111 (xo data sits at rows 64:112 of t2)
    alpha33 = float(w["eog_inw"][2, 0]) * float(w["eog_ow"][0, 0]) / 33.0
    beta = float(w["eog_inb"][2]) * float(w["eog_ow"][0, 0]) + float(w["eog_ob"][0])
    cwo = w["eog_cw"].astype(np.float64)             # [32,1,33]
    raw_blk = np.zeros((128, 32), np.float64)
    alp_blk = np.zeros((128, 32), np.float64)
    for l in range(33):
        raw_blk[64 + l, :] = cwo[:, 0, l]
        alp_blk[64 + l, :] = alpha33
    alp_blk[64 + 33, :] = beta                       # host plants 1.0 in xo[:,33]
    wbf[:, ogr_off: ogr_off + 32] = raw_blk
    wbf[:, ogo_off: ogo_off + 32] = alp_blk

    # fused (ef,of)->feat weights, elu stack rows: [eeg0, eeg1, og0, og1]
    # (eog rows grouped at 64:128 so the og copy+mul cover both halves at once)
    fw = w["fus_w"].astype(np.float64)               # [64,128]
    W_e = fw[:, :64] @ w["eeg_fw"].astype(np.float64)    # [64,32]
    W_o = fw[:, 64:] @ w["eog_fw"].astype(np.float64)    # [64,32]
    elu_blk = np.zeros((128, 128), np.float64)
    elu_blk[0:32, 0:64] = W_e.T
    elu_blk[32:64, 64:128] = W_e.T
    elu_blk[64:96, 0:64] = W_o.T
    elu_blk[96:128, 64:128] = W_o.T
    wbf[:, elu_off: elu_off + 128] = elu_blk

    # --- f32 blob: matmul blocks first, bias columns after ---
    cols = {}
    blocks = []

    def add(name, arr):                              # arr [128, n]
        cols[name] = sum(b.shape[1] for b in blocks)
        blocks.append(arr)

    def bdiag(blk):                                  # block-diag [128,128]
        out = np.zeros((128, 128))
        out[0:64, 0:64] = blk
        out[64:128, 64:128] = blk
        return out

    add("meanW", bdiag(np.full((64, 64), 1.0 / 64.0)))
    add("center", bdiag(np.eye(64) - 1.0 / 64.0))

    pe0 = (np.arange(64) % 2).astype(np.float64)
    b_feat = (fw[:, :64] @ w["eeg_fb"].astype(np.float64)
              + fw[:, 64:] @ w["eog_fb"].astype(np.float64)
              + w["fus_b"].astype(np.float64) + pe0
              - W_e.sum(axis=1) - W_o.sum(axis=1))   # fold elu's (e'-1)

    lay = []
    for i in range(2):
        s1 = w["tl_ln1_s"][i].astype(np.float64)
        b1v = w["tl_ln1_b"][i].astype(np.float64)
        Wvo = w["tl_ow"][i].astype(np.float64) @ w["tl_inw"][i, 128:192].astype(np.float64)
        bvo = (w["tl_ow"][i].astype(np.float64) @ w["tl_inb"][i, 128:192].astype(np.float64)
               + w["tl_ob"][i].astype(np.float64))
        Wvo_s = Wvo * s1[None, :]
        bvo_t = Wvo @ b1v + bvo
        s2 = w["tl_ln2_s"][i].astype(np.float64)
        b2v = w["tl_ln2_b"][i].astype(np.float64)
        W1 = w["tl_w1"][i].astype(np.float64)        # [256,64]
        W1_s = W1 * s2[None, :]
        b1g = W1 @ b2v + w["tl_b1"][i].astype(np.float64)   # [256]
        W2 = w["tl_w2"][i].astype(np.float64)        # [64,256]
        b2c = w["tl_b2"][i].astype(np.float64)
        lay.append((Wvo_s, bvo_t, W1_s, b1g, W2, b2c))
        add(f"attn{i}", bdiag(Wvo_s.T))
        m1 = W1_s.T                                  # [64,256]
        add(f"mlp1_{i}", np.concatenate([m1, m1], axis=0))
        # mlp2 weights go to the bf16 blob (bf16 matmul has no fp32r
        # dst-pattern restriction for partition-offset outputs)
        wbf[:, mlp2_off + 128 * i: mlp2_off + 128 * (i + 1)] = (
            np.concatenate([W2.T[0:128], W2.T[128:256]], axis=1))

    fn_s = w["fn_s"].astype(np.float64)
    fn_b = w["fn_b"].astype(np.float64)
    cls_w = w["cls_w"].astype(np.float64)
    cls_s = cls_w * fn_s[None, :]                    # [3,64]
    b_cls = cls_w @ fn_b + w["cls_b"].astype(np.float64)
    csT = cls_s.T                                    # [64,3]
    clsblk = np.zeros((128, 67))
    clsblk[0:64, 0:3] = csT
    clsblk[64:128, 64:67] = csT
    wbf[:, cls_off: cls_off + 67] = clsblk           # bf16 blob (67-row out)

    mm_cols = sum(b.shape[1] for b in blocks)        # float32r boundary

    # per-partition bias columns (float32)
    def col(vals128):
        return np.asarray(vals128, np.float64).reshape(128, 1)

    cb_e = w["eeg_cb"].astype(np.float64)
    cb_o = w["eog_cb"].astype(np.float64)
    add("cbcol", col(np.concatenate([cb_e, cb_e, cb_o, cb_o])))
    add("bfeat", col(np.concatenate([b_feat, b_feat])))
    for i in range(2):
        add(f"bvo{i}", col(np.concatenate([lay[i][1], lay[i][1]])))
        add(f"b1gA{i}", col(lay[i][3][0:128]))
        add(f"b1gB{i}", col(lay[i][3][128:256]))
        add(f"b2c{i}", col(np.concatenate([lay[i][5], lay[i][5]])))
    bc = np.zeros(128)
    bc[0:3] = b_cls
    bc[64:67] = b_cls
    add("bcls", col(bc))
    add("eps", col(np.full(128, 1e-5)))
    add("zero", col(np.zeros(128)))

    wf32 = np.concatenate(blocks, axis=1)
    off = {"ob": ob_off, "h": h_off, "ogr": ogr_off, "ogo": ogo_off,
           "elu": elu_off, "mlp2": mlp2_off, "cls": cls_off,
           "mm_cols": mm_cols}
    return wbf.astype(BF16), wf32.astype(F32), cols, off


# ---------------------------------------------------------------- device build
_CACHE = {}


def _build(nbf, nf32, cols, off):
    mm_cols = off["mm_cols"]
    ncol = nf32 - mm_cols
    nc = bacc.Bacc("TRN2", target_bir_lowering=False, debug=False)
    xe_d = nc.dram_tensor("xe", [B_CORE, XE_COLS], dt.bfloat16, kind="ExternalInput")
    wbf_d = nc.dram_tensor("wbf", [128, nbf], dt.bfloat16, kind="ExternalInput")
    wf_d = nc.dram_tensor("wf32", [128, nf32], dt.float32, kind="ExternalInput")
    y_d = nc.dram_tensor("y_fm", [6, N_PAIR * SUB], dt.float32, kind="ExternalOutput")

    # persistent sbuf arrays
    wbf_sb = nc.alloc_sbuf_tensor("wbf_sb", [128, nbf], dt.bfloat16).ap()
    wmm_sb = nc.alloc_sbuf_tensor("wmm_sb", [128, mm_cols], dt.float32r).ap()
    wcol_sb = nc.alloc_sbuf_tensor("wcol_sb", [128, ncol], dt.float32).ap()
    featA = nc.alloc_sbuf_tensor("featA", [128, N_PAIR * SUB], dt.float32r).ap()
    featB = nc.alloc_sbuf_tensor("featB", [128, N_PAIR * SUB], dt.float32r).ap()
    xn_sb = nc.alloc_sbuf_tensor("xn_sb", [128, N_PAIR * SUB], dt.float32r).ap()
    out_fm = nc.alloc_sbuf_tensor("out_fm", [128, N_PAIR * SUB], dt.float32).ap()

    def W(name, r0=0, r1=128, n=None):
        c0 = cols[name]
        if n is None:
            n = {"meanW": 128, "center": 128}.get(name, 64)
        return wmm_sb[r0:r1, c0:c0 + n]

    def Wc(name, r0=0, r1=128):                      # bias columns
        c0 = cols[name] - mm_cols
        return wcol_sb[r0:r1, c0:c0 + 1]

    with tile.TileContext(nc) as tc:
        nc.sync.dma_start(wbf_sb, wbf_d.ap())
        nc.sync.dma_start(wmm_sb, wf_d.ap()[:, 0:mm_cols].bitcast(dt.float32r))
        nc.sync.dma_start(wcol_sb, wf_d.ap()[:, mm_cols:nf32])

        # =================== PASS 1: front end (ACT set: exp) ===================
        with tc.tile_pool(name="p1sb", bufs=3) as sb, \
             tc.tile_pool(name="p1psA", bufs=2, space="PSUM") as psA, \
             tc.tile_pool(name="p1ps", bufs=1, space="PSUM") as ps:
            for p in range(N_PAIR):
                psH = psA.tile([128, SUB], dt.float32, tag="H")
                psF = ps.tile([128, SUB], dt.float32, tag="F")
                psR = ps.tile([128, SUB], dt.float32, tag="ogr")
                psO = ps.tile([128, SUB], dt.float32, tag="ogo")
                for h in range(2):
                    s = 2 * p + h
                    t0 = sb.tile([128, SUB], dt.bfloat16, tag="t0")
                    t1 = sb.tile([128, SUB], dt.bfloat16, tag="t1")
                    t2 = sb.tile([128, SUB], dt.bfloat16, tag="t2")
                    for t, tt in enumerate((t0, t1, t2)):
                        nc.sync.dma_start_transpose(
                            tt, xe_d.ap()[s * SUB:(s + 1) * SUB, 128 * t:128 * (t + 1)])
                    # obar (incl c_e bias via data ones-row)
                    psOB = psA.tile([128, SUB], dt.float32, tag="ob")
                    for t, tt in enumerate((t0, t1, t2)):
                        k = 64 if t == 2 else 128
                        nc.tensor.matmul(
                            psOB, wbf_sb[0:k, off["ob"] + 128 * t: off["ob"] + 128 * t + 128],
                            tt[0:k], start=(t == 0), stop=(t == 2))
                    obar2 = sb.tile([128, SUB], dt.bfloat16, tag="obar")
                    nc.scalar.activation(obar2, psOB, AF.Identity, bias=Wc("zero"))
                    # y = x * obar  (all-bf16 -> DVE 2x mode)
                    y0 = sb.tile([128, SUB], dt.bfloat16, tag="y0")
                    y1 = sb.tile([128, SUB], dt.bfloat16, tag="y1")
                    y2 = sb.tile([128, SUB], dt.bfloat16, tag="y2")
                    nc.gpsimd.tensor_mul(y0, t0, obar2)
                    nc.gpsimd.tensor_mul(y1, t1, obar2)
                    nc.vector.tensor_mul(y2[0:64], t2[0:64], obar2[0:64])
                    # h (eeg) -> psH rows [0:32] / [32:64]
                    o = 32 * h
                    for t, yy in enumerate((y0, y1, y2)):
                        k = 64 if t == 2 else 128
                        nc.tensor.matmul(
                            psH[o:o + 32], wbf_sb[0:k, off["h"] + 32 * t: off["h"] + 32 * t + 32],
                            yy[0:k], start=(t == 0), stop=(t == 2))
                    # eog raw & alpha terms -> rows [64:96] / [96:128]
                    o2 = 64 + 32 * h
                    nc.tensor.matmul(psR[o2:o2 + 32], wbf_sb[64:112, off["ogr"]:off["ogr"] + 32],
                                     t2[64:112], tile_position=(64, o2))
                    nc.tensor.matmul(psO[o2:o2 + 32], wbf_sb[64:112, off["ogo"]:off["ogo"] + 32],
                                     t2[64:112], tile_position=(64, o2))
                # og = raw * (alpha*mean+beta): one copy + one mul cover both halves
                o2sb = sb.tile([128, SUB], dt.float32, tag="o2sb")
                nc.vector.tensor_copy(o2sb[64:128], psO[64:128])
                nc.vector.tensor_mul(psH[64:128], psR[64:128], o2sb[64:128])
                # elu on stacked [eeg0, eeg1, og0, og1]
                r1 = sb.tile([128, SUB], dt.bfloat16, tag="r1")
                sm = sb.tile([128, SUB], dt.bfloat16, tag="sm")
                e1 = sb.tile([128, SUB], dt.bfloat16, tag="e1")
                eluT = sb.tile([128, SUB], dt.bfloat16, tag="elu")
                nc.vector.tensor_scalar(r1, psH, Wc("cbcol"), 0.0, ALU.add, ALU.max)
                nc.vector.tensor_scalar(sm, psH, Wc("cbcol"), 0.0, ALU.add, ALU.min)
                nc.scalar.activation(e1, sm, AF.Exp, bias=Wc("zero"))
                nc.vector.tensor_add(eluT, r1, e1)
                # feat = W_elu.T @ elu (+b_feat), block-diag over halves
                nc.tensor.matmul(psF, wbf_sb[:, off["elu"]:off["elu"] + 128], eluT)
                nc.scalar.activation(featA[:, p * SUB:(p + 1) * SUB], psF,
                                     AF.Identity, bias=Wc("bfeat"))

        # =================== PASS 2: transformer ===================
        def layer_norm(ps, sb, src_ap, xn_out_ap, dev=False):
            """src [128,SUB] sbuf -> xn = (x-mean)/sqrt(var+eps).

            dev=True: deviation form — mean via matmul, d = x - mean on DVE
            (frees the PSUM bank fast), square + final scale on the Pool
            engine (SBUF-only).  dev=False: center form — centered values
            via matmul, square on ACT, scale on DVE.  Both get rstd in one
            Abs_reciprocal_sqrt (1/64 is folded into the meanW weights).
            """
            if dev:
                psM = ps.tile([128, SUB], dt.float32, tag="m")
                nc.tensor.matmul(psM, W("meanW"), src_ap)
                d = sb.tile([128, SUB], dt.float32, tag="d")
                nc.vector.tensor_sub(d, src_ap, psM)
                sqd = sb.tile([128, SUB], dt.float32r, tag="sq")
                nc.gpsimd.tensor_mul(sqd, d, d)
                psV = ps.tile([128, SUB], dt.float32, tag="v")
                nc.tensor.matmul(psV, W("meanW"), sqd)
                rstd = sb.tile([128, SUB], dt.float32, tag="rs")
                nc.scalar.activation(rstd, psV, AF.Abs_reciprocal_sqrt,
                                     bias=Wc("eps"))
                nc.gpsimd.tensor_mul(xn_out_ap, d, rstd)
            else:
                psXC = ps.tile([128, SUB], dt.float32, tag="xc")
                nc.tensor.matmul(psXC, W("center"), src_ap)
                sq = sb.tile([128, SUB], dt.float32r, tag="sq")
                nc.scalar.activation(sq, psXC, AF.Square, bias=Wc("zero"))
                psV = ps.tile([128, SUB], dt.float32, tag="v")
                nc.tensor.matmul(psV, W("meanW"), sq)
                rstd = sb.tile([128, SUB], dt.float32, tag="rs")
                nc.scalar.activation(rstd, psV, AF.Abs_reciprocal_sqrt,
                                     bias=Wc("eps"))
                nc.vector.tensor_mul(xn_out_ap, psXC, rstd)

        fsrc, fdst = featA, featB
        for i in range(2):
            # ---- wave 1: LN1 + attn + LN2  (ACT set: abs_rsqrt) ----
            tc.no_sync_barrier()
            with tc.tile_pool(name=f"w1s{i}", bufs=4) as sb, \
                 tc.tile_pool(name=f"w1c{i}", bufs=2, space="PSUM") as psc, \
                 tc.tile_pool(name=f"w1p{i}", bufs=2, space="PSUM") as ps:
                # two pairs in flight: interleave the LN/attn stages so the
                # in-order engine queues always have independent work between
                # dependent hops of one pair's chain.
                def w1_stageA(p):
                    """LN1 (dev form) stats front: mean + d + sq + var."""
                    sl = slice(p * SUB, (p + 1) * SUB)
                    psM = ps.tile([128, SUB], dt.float32, tag="m")
                    nc.tensor.matmul(psM, W("meanW"), fsrc[:, sl])
                    d = sb.tile([128, SUB], dt.float32, tag="d")
                    nc.vector.tensor_sub(d, fsrc[:, sl], psM)
                    sqd = sb.tile([128, SUB], dt.float32r, tag="sq")
                    nc.gpsimd.tensor_mul(sqd, d, d)
                    psV = ps.tile([128, SUB], dt.float32, tag="v")
                    nc.tensor.matmul(psV, W("meanW"), sqd)
                    return sl, d, psV

                def w1_stageB(st):
                    """rstd + xn + attn matmul + residual STT -> fdst."""
                    sl, d, psV = st
                    rstd = sb.tile([128, SUB], dt.float32, tag="rs")
                    nc.scalar.activation(rstd, psV, AF.Abs_reciprocal_sqrt,
                                         bias=Wc("eps"))
                    xn1 = sb.tile([128, SUB], dt.float32r, tag="xn1")
                    nc.gpsimd.tensor_mul(xn1, d, rstd)
                    psF2 = psc.tile([128, SUB], dt.float32, tag="c")
                    nc.tensor.matmul(psF2, W(f"attn{i}", n=128), xn1)
                    nc.vector.scalar_tensor_tensor(
                        fdst[:, sl], psF2, Wc(f"bvo{i}"), fsrc[:, sl],
                        ALU.add, ALU.add)
                    return sl

                def w1_stageC(sl):
                    """LN2 (center form) -> xn_sb."""
                    layer_norm(ps, sb, fdst[:, sl], xn_sb[:, sl])

                for p in range(0, N_PAIR, 2):
                    stA = w1_stageA(p)
                    stB = w1_stageA(p + 1)
                    slA = w1_stageB(stA)
                    slB = w1_stageB(stB)
                    w1_stageC(slA)
                    w1_stageC(slB)
            # ---- wave 2: MLP (ACT set: gelu) ----
            tc.no_sync_barrier()
            with tc.tile_pool(name=f"w2s{i}", bufs=5) as sb, \
                 tc.tile_pool(name=f"w2c{i}", bufs=2, space="PSUM") as psc, \
                 tc.tile_pool(name=f"w2p{i}", bufs=5, space="PSUM") as ps:
                for p in range(N_PAIR):
                    sl = slice(p * SUB, (p + 1) * SUB)
                    gsb = []
                    for hh in range(2):              # half: lo/hi
                        for ch in range(2):          # hidden chunk A/B
                            gp = ps.tile([128, SUB], dt.float32, tag="g")
                            nc.tensor.matmul(
                                gp, W(f"mlp1_{i}", 64 * hh, 64 * hh + 64, n=256)
                                    [:, 128 * ch:128 * ch + 128],
                                xn_sb[64 * hh:64 * hh + 64, sl])
                            g = sb.tile([128, SUB], dt.bfloat16, tag=f"g{hh}{ch}")
                            nc.scalar.activation(g, gp, AF.Gelu,
                                                 bias=Wc(f"b1g{'AB'[ch]}{i}"))
                            gsb.append(g)
                    psF3 = psc.tile([128, SUB], dt.float32, tag="c")
                    m2o = off["mlp2"] + 128 * i
                    for hh in range(2):
                        for ch in range(2):
                            nc.tensor.matmul(
                                psF3[64 * hh:64 * hh + 64],
                                wbf_sb[:, m2o + 64 * ch: m2o + 64 * ch + 64],
                                gsb[2 * hh + ch], start=(ch == 0),
                                stop=(ch == 1))
                    nc.vector.scalar_tensor_tensor(
                        fsrc[:, sl], psF3, Wc(f"b2c{i}"), fdst[:, sl],
                        ALU.add, ALU.add)
            # after layer: result lives in fsrc again (A -> B -> A)

        # ---- wave 3: final LN + classifier (ACT set: abs_rsqrt) ----
        tc.no_sync_barrier()
        with tc.tile_pool(name="w3s", bufs=4) as sb, \
             tc.tile_pool(name="w3c", bufs=2, space="PSUM") as psc, \
             tc.tile_pool(name="w3p", bufs=3, space="PSUM") as ps:
            def w3_stageA(p):
                sl = slice(p * SUB, (p + 1) * SUB)
                psXC = ps.tile([128, SUB], dt.float32, tag="xc")
                nc.tensor.matmul(psXC, W("center"), fsrc[:, sl])
                sq = sb.tile([128, SUB], dt.float32r, tag="sq")
                nc.scalar.activation(sq, psXC, AF.Square, bias=Wc("zero"))
                psV = ps.tile([128, SUB], dt.float32, tag="v")
                nc.tensor.matmul(psV, W("meanW"), sq)
                return sl, psXC, psV

            def w3_stageB(st):
                sl, psXC, psV = st
                rstd = sb.tile([128, SUB], dt.float32, tag="rs")
                nc.scalar.activation(rstd, psV, AF.Abs_reciprocal_sqrt,
                                     bias=Wc("eps"))
                xn3 = sb.tile([128, SUB], dt.bfloat16, tag="xn3")
                nc.vector.tensor_mul(xn3, psXC, rstd)
                psO = psc.tile([128, SUB], dt.float32, tag="c")
                nc.tensor.matmul(psO[0:67],
                                 wbf_sb[:, off["cls"]: off["cls"] + 67], xn3)
                nc.vector.tensor_scalar_add(out_fm[0:67, sl], psO[0:67],
                                            Wc("bcls", 0, 67))

            for p in range(0, N_PAIR, 2):
                stA = w3_stageA(p)
                stB = w3_stageA(p + 1)
                w3_stageB(stA)
                w3_stageB(stB)
        nc.sync.dma_start(y_d.ap()[0:3, :], out_fm[0:3, :])
        nc.sync.dma_start(y_d.ap()[3:6, :], out_fm[64:67, :])

    nc.compile()
    return nc


# ---------------------------------------------------------------- entry point
def kernel(**inputs):
    w = {k: np.asarray(v) for k, v in inputs.items()}
    wbf, wf32, cols, off = _fold_weights(w)

    # x320 l-major: xe[:, 64*l + c] = eeg[:, c, l]; col 62 = 1.0 (c_e bias row)
    eeg = w["eeg"].astype(F32)
    xe = np.zeros((B_TOTAL, XE_COLS), F32)
    xe[:, 0:320].reshape(B_TOTAL, 5, 64)[:, :, 0:62] = eeg.transpose(0, 2, 1)
    xe[:, 62] = 1.0
    xe[:, 320:353] = w["eog"].astype(F32)[:, 0, :]
    xe[:, 353] = 1.0                                 # beta bias row (xo row 33)
    xe = xe.astype(BF16)

    key = ("prog", wbf.shape[1], wf32.shape[1])
    if key not in _CACHE:
        _CACHE[key] = _build(wbf.shape[1], wf32.shape[1], cols, off)
    nc = _CACHE[key]

    in_maps = []
    for k in range(N_CORES):
        in_maps.append({
            "xe": np.ascontiguousarray(xe[k * B_CORE:(k + 1) * B_CORE]),
            "wbf": wbf, "wf32": wf32,
        })
    res = run_bass_kernel_spmd(nc, in_maps, core_ids=list(range(N_CORES)))

    out = np.empty((B_TOTAL, 3), F32)
    for k in range(N_CORES):
        y = res.results[k]["y_fm"].reshape(2, 3, N_PAIR, SUB)
        out[k * B_CORE:(k + 1) * B_CORE] = (
            y.transpose(2, 0, 3, 1).reshape(B_CORE, 3))
    return out


if __name__ == "__main__":
    import reference
    ins = {k: np.asarray(v) for k, v in reference.setup_inputs().items()}
    got = kernel(**ins)
    exp = np.asarray(reference.reference(**ins))
    err = np.abs(got - exp).max() / (np.abs(exp).max() + 1e-9)
    print("Relative error:", err)
